# revision 6
# baseline (speedup 1.0000x reference)
"""DuelingDQN forward for 8 Trainium2 NeuronCores — pure batch data-parallel.

Per batch element b (reference semantics):
  market = state[b, :, :64]; port = state[b, 179, 64:]
  Q_h = market @ Wq_h.T + bq_h ; K_h likewise          (4 heads of 16)
  E_h = Q_h @ K_h.T / 4 ; P_h = softmax(E_h, -1)       (|E| small: skip max-sub)
  att = concat_h(mean_qs(P_h) @ V_h) @ Wo.T + bo       (V_h = market @ Wv_h.T)
  combined = [att, port] -> MLP dueling head -> out [3]

v2 restructurings on top of the v1 kernel:
  * E-tile PSUM packs 2 heads per bank at 180-col offsets so ONE exp
    activation covers a whole E-tile (4 heads, 720 cols): 3 ACT instrs
    per pair-group instead of 6, amortizing the ~185ns ACT access tax.
  * softmax row-sums via a DVE fold chain (two bf16 2x tensor_tensor adds
    halving 180->90->45, then a 1x tensor_reduce) instead of one full-width
    1x reduce: ~1.6us vs 2.3us per pair-group.
  * meanP via THIN-stationary matmuls: stationary = w-masked recip columns
    [128,2] (2-col LDWEIGHTS) with exp moving, output [2,180] packed at
    32h partition bases in one PSUM bank; reorientation to k-partitions by
    two DMA xbar transposes per group.  Replaces 24 fat-LDW (116-col)
    matmuls per group with 12 thin ones + transposes on idle DMA queues.
  * the dueling MLP head runs ONCE per core over all 256 batch elements
    (moving N=256, bf16 weights) instead of once per 8 — removes ~1500
    tiny matmuls and their weight reloads.
  * Wo/bo folded into W1 host-side; q/k bias via appended constant-1 row;
    bf16 on every high-volume PE path.
"""

from contextlib import ExitStack

import numpy as np

S, F, MKT, H, HD, ATT = 180, 68, 64, 4, 16, 64
FC1, FC2, NACT = 256, 128, 3
B_TOT, NCORES = 2048, 8
BC = B_TOT // NCORES

_CACHE = {}


def _bf16(x):
    import ml_dtypes
    return np.asarray(x, np.float32).astype(ml_dtypes.bfloat16)


def _group_masks():
    """r-column masks for one 2-batch group (3 E-tiles).

    rbuf layout per group: [3 tiles, 4 heads, 2 bsel] = 24 cols.  bsel 0/1 =
    which batch element of the pair a column weights; rows not belonging to
    that element (including junk rows) are zeroed.  Tiles hold qs rows:
      tile0: b-even 0:128 | tile1: b-even 128:180(+junk), b-odd 0:64
      tile2: b-odd 64:180(+junk)
    """
    ones = np.ones(128, np.float32)
    z = np.zeros(128, np.float32)
    m52 = z.copy(); m52[0:52] = 1
    m64h = z.copy(); m64h[64:128] = 1
    m116 = z.copy(); m116[0:116] = 1
    sel = {(0, 0): ones, (0, 1): z, (1, 0): m52, (1, 1): m64h,
           (2, 0): z, (2, 1): m116}
    mask = np.zeros((128, 3, H, 2), np.float32)
    for t in range(3):
        for b in range(2):
            mask[:, t, :, b] = sel[(t, b)][:, None]
    return mask.reshape(128, 24)


def _host_prep(inp):
    f32 = lambda x: np.ascontiguousarray(x, np.float32)
    Wq, Wk, Wv, Wo = (np.asarray(inp[k], np.float32) for k in ("Wq", "Wk", "Wv", "Wo"))
    bq, bk, bo, bv = (np.asarray(inp[k], np.float32) for k in ("bq", "bk", "bo", "bv"))

    # Stationary for Q/K projection: [65, 128].  K-dim = 64 market features
    # + one constant-1 row carrying the bias.  M-dim = 4 heads x 32 (16 real
    # dims + 16 zero pad so each head sits on a 32-aligned partition block).
    # qT partition blocks carry heads in order (h0, h2, h1, h3): the E-tile
    # contracts 64 rows (one PAIR of head blocks) so the two heads sharing a
    # PSUM bank share a PE row-group and serialize in the array (a PSUM bank
    # may not be written by two concurrent row-tiled matmuls).  The K side is
    # projected into two half-zero "spaces": space A holds h0/h1 data (zeros
    # at the partner blocks), space B holds h2/h3.
    ROWH = {0: 0, 2: 1, 1: 2, 3: 3}        # head -> 32-row block in qT/kT
    lq = np.zeros((MKT + 1, 128), np.float32)
    lk = np.zeros((MKT + 1, 2, 128), np.float32)
    for h in range(H):
        rb = 32 * ROWH[h]
        lq[:MKT, rb:rb + HD] = Wq[HD * h:HD * h + HD, :].T
        lq[MKT, rb:rb + HD] = bq[HD * h:HD * h + HD]
        sp = 0 if h in (0, 1) else 1
        lk[:MKT, sp, rb:rb + HD] = Wk[HD * h:HD * h + HD, :].T
        lk[MKT, sp, rb:rb + HD] = bk[HD * h:HD * h + HD]

    W1, b1 = np.asarray(inp["W1"], np.float32), np.asarray(inp["b1"], np.float32)
    W1a, W1p = W1[:, :ATT], W1[:, ATT:]
    W1e = (W1a @ Wo).T                                         # [64, 256]
    # att rows live at partition 32h+d (16 real + 16 zero pad per head) so the
    # per-head copies land on 32-aligned partition bases.
    W1cT = np.zeros((128, FC1), np.float32)
    for h in range(H):
        W1cT[32 * h:32 * h + HD] = W1e[HD * h:HD * h + HD]
    # bv enters att exactly: the softmax-mean weights sum to 1, so
    # att = mbar @ Wv.T + bv  ->  fold (bo + Wo @ bv) through W1a.
    b1eff = b1 + W1a @ (bo + Wo @ bv)
    b1c = f32(b1eff.reshape(2, 128).T)                         # [128, 2]

    def noisy(p):
        W = inp[f"{p}_wmu"] + inp[f"{p}_wsig"] * inp[f"{p}_weps"]
        b = inp[f"{p}_bmu"] + inp[f"{p}_bsig"] * inp[f"{p}_beps"]
        return np.asarray(W, np.float32), np.asarray(b, np.float32)

    v1W, v1b = noisy("v1"); v2W, v2b = noisy("v2")
    a1W, a1b = noisy("a1"); a2W, a2b = noisy("a2")
    gmask = _group_masks()

    consts = {
        "lq": _bf16(lq), "lk": _bf16(lk),
        # 1/S folds the mean-over-query-positions into the Wv application
        "WvT": f32(Wv.T / S),
        "W1cT": _bf16(W1cT), "W1pT": _bf16(W1p.T), "b1c": b1c,
        "v1T": _bf16(v1W.T.reshape(2, 128, FC2).transpose(1, 0, 2)),  # [128,2,128]
        "a1T": _bf16(a1W.T.reshape(2, 128, FC2).transpose(1, 0, 2)),
        "v2T": _bf16(v2W.T), "a2T": _bf16(a2W.T),
        "bv1": f32(v1b.reshape(FC2, 1)), "ba1": f32(a1b.reshape(FC2, 1)),
        # v2 bias folded in: out = adv + (v - mean(adv)) + bv2 + (ba2 - mean(ba2))
        "ba2c": f32((a2b - a2b.mean() + v2b.reshape(-1)[0]).reshape(NACT, 1)),
        "ident": f32(np.eye(128)),
        "ones3": _bf16(np.full((NACT, 1), 1.0 / 3.0)),
        "gmask": _bf16(np.tile(gmask, (1, 4))),                # [128, 4*24]
    }
    return consts, float(v2b.reshape(-1)[0])


# E-tiles per 2-batch group: each covers 128 consecutive qT columns of the
# pair's 384-col block (tile1 straddles the two batch elements):
#   tile0 = b-even qs 0:128 | tile1 = b-even 128:192, b-odd 0:64
#   tile2 = b-odd 64:192     (cols 180:192 of each element are junk pad)


def build_nc(bc=BC, nb=32, v2b=0.0, nch_limit=None, repeat=1, stage=99):
    import concourse.bacc as bacc
    import concourse.tile as tile
    from concourse import mybir

    fp32 = mybir.dt.float32
    bf16 = mybir.dt.bfloat16
    AF = mybir.ActivationFunctionType
    ALU = mybir.AluOpType
    AX = mybir.AxisListType

    assert nb % 8 == 0 and bc % nb == 0
    nch, ngrp = bc // nb, nb // 2
    nch_run = nch if nch_limit is None else min(nch, nch_limit)
    SP2 = 192      # per-b column stride in mktT/qT/kT: 180 real + 12 junk.
    ncols = nb * SP2

    nc = bacc.Bacc(None, target_bir_lowering=False)
    st = nc.dram_tensor("state_c", [bc, S, F], fp32, kind="ExternalInput")
    out_d = nc.dram_tensor("out_c", [bc, NACT], fp32, kind="ExternalOutput")

    cshape = {
        "lq": ([MKT + 1, 128], bf16), "lk": ([MKT + 1, 2, 128], bf16),
        "WvT": ([ATT, ATT], fp32),
        "W1cT": ([128, FC1], bf16), "W1pT": ([4, FC1], bf16),
        "b1c": ([128, 2], fp32),
        "v1T": ([128, 2, FC2], bf16), "a1T": ([128, 2, FC2], bf16),
        "v2T": ([FC2, 1], bf16), "a2T": ([FC2, NACT], bf16),
        "bv1": ([FC2, 1], fp32), "ba1": ([FC2, 1], fp32),
        "ba2c": ([NACT, 1], fp32),
        "ident": ([128, 128], fp32), "ones3": ([NACT, 1], bf16),
        "gmask": ([128, 96], bf16),
    }
    dts = {k: nc.dram_tensor(k, shp, dt, kind="ExternalInput")
           for k, (shp, dt) in cshape.items()}

    with tile.TileContext(nc) as tc, ExitStack() as ctx:
        constp = ctx.enter_context(tc.tile_pool(name="const", bufs=1))
        mktp = ctx.enter_context(tc.tile_pool(name="mktT", bufs=2))
        qktp = ctx.enter_context(tc.tile_pool(name="qkT", bufs=2))
        rsp = ctx.enter_context(tc.tile_pool(name="rs", bufs=5))
        zfp = ctx.enter_context(tc.tile_pool(name="zf", bufs=3))
        smallp = ctx.enter_context(tc.tile_pool(name="small", bufs=2))
        ptbp = ctx.enter_context(tc.tile_pool(name="ptb", bufs=3))
        mkscp = ctx.enter_context(tc.tile_pool(name="mkscp", bufs=1))
        mptp = ctx.enter_context(tc.tile_pool(name="mpt", bufs=4))
        # PSUM: "ps" holds E-tiles [128, 2, 512] (2 heads per bank at
        # 180-col offsets, 2 banks per tile), 3 slots pipeline PE->ACT.
        # "psq" (1 bank, 2 slots) serves projection, meanPT and the MLP.
        psp = ctx.enter_context(tc.tile_pool(name="ps", bufs=3, space="PSUM"))
        psq = ctx.enter_context(tc.tile_pool(name="psq", bufs=2, space="PSUM"))

        cst = {}
        for k, (shp, dt) in cshape.items():
            t = constp.tile(shp, dt, tag=k, name=k + "_sb")
            nc.sync.dma_start(out=t[:], in_=dts[k][:])
            cst[k] = t
        outT = constp.tile([NACT, bc], fp32, tag="outT")
        combG = constp.tile([128, bc], bf16, tag="combG")
        portF = constp.tile([4, bc], fp32, tag="portF")
        portT = constp.tile([4, bc], bf16, tag="portT")
        nc.vector.memset(combG[:], 0.0)
        # expE ring: 24 slots in ONE contiguous tile; slot = [4 heads, 180]
        exring = constp.tile([128, 24, H * S], bf16, tag="exring")
        # double-buffered bf16 market staging (s-major), loaded by casting
        # gpsimd DMA; cols 64:128 are the constant bias-1 row + zero pad,
        # written once.
        mkbs = []
        for half in "AB":
            m0 = constp.tile([128, nb, 128], bf16, tag=f"mkb0{half}")
            m1 = constp.tile([64, nb, 128], bf16, tag=f"mkb1{half}")
            for m in (m0, m1):
                nc.vector.memset(m[:, :, MKT:], 0.0)
                nc.vector.memset(m[:, :, MKT:MKT + 1], 1.0)
            mkbs.append((m0, m1))

        def stage_b1_gl(rbases, rbuf, gl, mptTg, ptbbA, ptbbB):
            # meanP: thin-stationary matmuls -> pt[(2h+b) rows, k]
            rbase = rbases[gl]
            pt = psq.tile([128, 512], fp32, tag="psq", name="pt")
            for h in range(H):
                for t3 in range(3):
                    exv = exring[:, rbase + t3, :].rearrange(
                        "p (h k) -> p h k", h=H)
                    nc.tensor.matmul(
                        pt[32 * h:32 * h + 2, 0:180],
                        rbuf[:, gl, t3, h, :],
                        exv[:, h, :],
                        start=(t3 == 0), stop=(t3 == 2),
                        tile_position=(0, 32 * h))
            nc.vector.tensor_copy(ptbbA[:, gl, :], pt[:, 0:128])
            nc.vector.tensor_copy(ptbbB[:, gl, 0:64], pt[:, 116:180])

        def stage_b1_end(mptTg, ptbbA, ptbbB):
            # reorient to k-partitions via two batched DMA transposes
            # (slot0 = k 0:128, slot1 = k 116:180; ptbbB cols 64:128 are
            # junk -> land on unused out rows).
            nc.sync.dma_start(
                out=mptTg[:, :, 0, :],
                in_=ptbbA[:].rearrange("p a b -> p (a b)"), transpose=True)
            nc.sync.dma_start(
                out=mptTg[:, :, 1, :],
                in_=ptbbB[:].rearrange("p a b -> p (a b)"), transpose=True)

        def stage_b2(b0, gq, mptTg, mkb0, mkb1):
            # mbarT[mkt, 4h] per b = market_b^T @ meanPT_b
            SP = psp.tile([128, 512], fp32, tag="ps", name="SP")
            for b8 in range(8):
                b = 8 * gq + b8
                gl, db = b8 // 2, b8 % 2
                mpv = mptTg[:].rearrange("p g s (h b) -> p g s h b", b=32)
                nc.tensor.matmul(SP[0:MKT, 4 * b8:4 * b8 + 4],
                                 mkb0[0:116, b, 0:MKT],
                                 mpv[0:116, gl, 0, :, db],
                                 start=True, stop=False)
                nc.tensor.matmul(SP[0:MKT, 4 * b8:4 * b8 + 4],
                                 mkb1[:, b, 0:MKT],
                                 mpv[0:64, gl, 1, :, db],
                                 start=False, stop=True)
            mbS = smallp.tile([MKT, 32], fp32, tag="mbS")
            nc.vector.tensor_copy(mbS[:], SP[0:MKT, 0:32])

            # att_pre per head -> combG rows 32h:32h+16
            mbv = mbS[:].rearrange("p (b h) -> p b h", h=H)
            for h in range(H):
                nc.tensor.matmul(
                    SP[32 * h:32 * h + HD, 64:72],
                    cst["WvT"][:, HD * h:HD * h + HD],
                    mbv[:, :, h],
                    start=True, stop=True, tile_position=(0, 32 * h))
            cb = b0 + 8 * gq
            for h in range(H):
                nc.vector.tensor_copy(
                    combG[32 * h:32 * h + HD, cb:cb + 8],
                    SP[32 * h:32 * h + HD, 64:72])
            nc.sync.dma_start(
                out=portF[:, cb:cb + 8].rearrange("p (b o) -> p b o", o=1),
                in_=st[cb:cb + 8, 179:180, ATT:F].transpose([2, 0, 1]))

        prevq = None
        prev2 = None
        for rep in range(repeat):
            for ch in range(nch_run):
                b0 = ch * nb
                # ------- state load (s-major, casting gpsimd DMA) -----------
                mkb0, mkb1 = mkbs[ch % 2]
                nc.gpsimd.dma_start(
                    out=mkb0[:, :, 0:MKT],
                    in_=st[b0:b0 + nb, 0:128, 0:MKT].transpose([1, 0, 2]))
                nc.gpsimd.dma_start(
                    out=mkb1[:, :, 0:MKT],
                    in_=st[b0:b0 + nb, 116:180, 0:MKT].transpose([1, 0, 2]))

                mktT = mktp.tile([128, ncols], bf16, tag="mktT")
                mkview = mktT[:].rearrange("p (b c) -> p b c", c=SP2)
                nc.vector.memset(mkview[0:MKT + 1, :, 180:192], 0.0)
                mksc = mkscp.tile([128, nb, 64], bf16, tag="mksc")
                # batched xbar transposes: one instruction per staging tile
                # (out 3D: per-128-col source block b -> out[:, b, :])
                nc.sync.dma_start(
                    out=mkview[:, :, 0:128],
                    in_=mkb0[:].rearrange("p a b -> p (a b)"), transpose=True)
                # s 116:180 lands unaligned; bounce via aligned scratch
                nc.sync.dma_start(
                    out=mksc[:],
                    in_=mkb1[:].rearrange("p a b -> p (a b)"), transpose=True)
                nc.vector.tensor_copy(mkview[0:128, :, 128:180], mksc[:, :, 12:64])

                # ---------------- Q/K projection -----------------------------
                qT = qktp.tile([128, ncols], bf16, tag="qT")
                kT = qktp.tile([128, 2, ncols], bf16, tag="kT")
                for ci, c in enumerate(range(0, ncols, 512)):
                    pp = psq.tile([128, 512], fp32, tag="psq", name="pp")
                    nc.tensor.matmul(
                        pp[:], cst["lq"][0:MKT + 1, :],
                        mktT[0:MKT + 1, c:c + 512],
                        start=True, stop=True)
                    if ci % 3 == 2:
                        nc.scalar.copy(qT[:, c:c + 512], pp[:])
                    else:
                        nc.vector.tensor_copy(qT[:, c:c + 512], pp[:])
                    pk = psp.tile([128, 2, 512], fp32, tag="ps", name="pk")
                    for sp in range(2):
                        nc.tensor.matmul(
                            pk[:, sp, :], cst["lk"][0:MKT + 1, sp, :],
                            mktT[0:MKT + 1, c:c + 512],
                            start=True, stop=True)
                    if ci % 2 == 0:
                        nc.scalar.copy(kT[:, :, c:c + 512], pk[:])
                    else:
                        nc.vector.tensor_copy(kT[:, :, c:c + 512], pk[:])

                # ---------------- attention ---------------------------------
                for gq in range(ngrp // 4):          # 4 pair-groups = 8 b
                    rbuf = rsp.tile([128, 4, 3, H, 2], bf16, tag="rbuf")
                    if stage >= 4 and prevq is not None:
                        mptTg_p = mptp.tile([128, 4, 2, 128], bf16, tag="mptT")
                        ptbbA_p = ptbp.tile([128, 4, 128], bf16, tag="ptbbA")
                        ptbbB_p = ptbp.tile([128, 4, 128], bf16, tag="ptbbB")
                    rbases = []
                    for gl in range(4):
                        g = 4 * gq + gl
                        rbase = ((ch * (ngrp // 4) + gq) * 4 + gl) % 8 * 3
                        rbases.append(rbase)
                        bcol = 2 * g * SP2
                        kc0, kc1 = bcol, bcol + SP2
                        for t3 in range(3):
                            ep = psp.tile([128, 2, 512], fp32, tag="ps", name="ep")
                            # One matmul per row-group computes BOTH heads of
                            # a bank: the pair shares the identical stationary
                            # (the K zero-spaces mask the partner), so the
                            # moving operand concatenates the two spaces'
                            # k-windows (N=360) into one bank write.
                            for rg in range(2):
                                hr = slice(64 * rg, 64 * rg + 64)
                                eo = ep[:, rg, 0:360]
                                eov = eo.rearrange("p (s x) -> p s x", s=2)
                                if t3 == 0:
                                    nc.tensor.matmul(
                                        eov, qT[hr, bcol:bcol + 128],
                                        kT[hr, :, kc0:kc0 + 180],
                                        start=True, stop=True,
                                        tile_position=(64 * rg, 0))
                                elif t3 == 1:
                                    nc.tensor.matmul(
                                        eov[0:64],
                                        qT[hr, bcol + 128:bcol + 192],
                                        kT[hr, :, kc0:kc0 + 180],
                                        start=True, stop=True,
                                        tile_position=(64 * rg, 0))
                                    nc.tensor.matmul(
                                        eov[64:128],
                                        qT[hr, kc1:kc1 + 64],
                                        kT[hr, :, kc1:kc1 + 180],
                                        start=True, stop=True,
                                        tile_position=(64 * rg, 64))
                                else:
                                    nc.tensor.matmul(
                                        eov,
                                        qT[hr, kc1 + 64:kc1 + 192],
                                        kT[hr, :, kc1:kc1 + 180],
                                        start=True, stop=True,
                                        tile_position=(64 * rg, 0))
                            ex = exring[:, rbase + t3, :]
                            # exring head order (h0,h1,h2,h3) = offsets
                            # (bank*180 + slot*360)
                            nc.scalar.activation(
                                ex.rearrange("p (j b x) -> p b j x", j=2, b=2),
                                ep[:, :, 0:360].rearrange(
                                    "p b (j x) -> p b j x", j=2),
                                AF.Exp, scale=0.25)
                        if stage < 3:
                            continue
                        # softmax denominators: one bf16 2x fold 180->90,
                        # then a 1x reduce.  (t,h) collapses to stride 180.
                        exg = exring[:, rbase:rbase + 3, :].rearrange(
                            "p t (h k) -> p (t h) k", h=H)
                        zf1 = zfp.tile([128, 12, 90], bf16, tag="zf1")
                        zf2 = zfp.tile([128, 12, 45], bf16, tag="zf2")
                        eng1 = nc.vector if gl % 2 == 0 else nc.gpsimd
                        eng2 = nc.gpsimd if gl % 2 == 0 else nc.vector
                        eng1.tensor_tensor(
                            out=zf1[:], in0=exg[:, :, 0:90],
                            in1=exg[:, :, 90:180], op=ALU.add)
                        eng2.tensor_tensor(
                            out=zf2[:], in0=zf1[:, :, 0:45],
                            in1=zf1[:, :, 45:90], op=ALU.add)
                        rsum = rsp.tile([128, 12], bf16, tag="rsum")
                        with nc.allow_low_precision(reason="softmax denom bf16"):
                            nc.vector.tensor_reduce(
                                rsum[:], zf2[:], axis=AX.X, op=ALU.add)
                        # r[qs] = mask / rowsum: rbuf [128, gl, tile, h, bsel]
                        rec = rsp.tile([128, 12], bf16, tag="rec")
                        with nc.allow_low_precision(reason="softmax recip bf16"):
                            nc.vector.reciprocal(rec[:], rsum[:])
                        rec4 = rec[:].rearrange("p (t h) -> p t h", t=3)
                        gm = cst["gmask"][:].rearrange(
                            "p (g t h b) -> p g t h b", g=4, t=3, h=H)
                        for bs in range(2):
                            nc.vector.tensor_tensor(
                                out=rbuf[:, gl, :, :, bs], in0=rec4,
                                in1=gm[:, gl, :, :, bs], op=ALU.mult)
                        # interleave prev-gq meanP so the PE always has
                        # fresh E-tiles for ACT between pt bursts
                        if stage >= 4 and prevq is not None:
                            stage_b1_gl(prevq[2], prevq[3], gl,
                                        mptTg_p, ptbbA_p, ptbbB_p)
                    if stage >= 4 and prevq is not None:
                        stage_b1_end(mptTg_p, ptbbA_p, ptbbB_p)
                        if stage >= 5 and prev2 is not None:
                            stage_b2(*prev2)
                        prev2 = (prevq[0], prevq[1], mptTg_p,
                                 prevq[4], prevq[5])
                    prevq = (b0, gq, rbases, rbuf, mkb0, mkb1)
        if stage >= 4 and prevq is not None:
            mptTg_f = mptp.tile([128, 4, 2, 128], bf16, tag="mptT")
            ptbbA_f = ptbp.tile([128, 4, 128], bf16, tag="ptbbA")
            ptbbB_f = ptbp.tile([128, 4, 128], bf16, tag="ptbbB")
            for gl in range(4):
                stage_b1_gl(prevq[2], prevq[3], gl, mptTg_f, ptbbA_f, ptbbB_f)
            stage_b1_end(mptTg_f, ptbbA_f, ptbbB_f)
            if stage >= 5:
                if prev2 is not None:
                    stage_b2(*prev2)
                stage_b2(prevq[0], prevq[1], mptTg_f, prevq[4], prevq[5])

        if stage < 5:
            nc.vector.memset(combG[:], 0.0)
            nc.vector.memset(portF[:], 0.0)
        # ---------------- dueling MLP head, batched over all bc ------------
        nc.vector.tensor_copy(portT[:], portF[:])
        ft = constp.tile([128, 2, bc], bf16, tag="ft")
        for hf in range(2):
            SPm = psq.tile([128, 512], fp32, tag="psq", name="SPm")
            nc.tensor.matmul(SPm[:, 0:bc], cst["W1cT"][:, 128 * hf:128 * hf + 128],
                             combG[:], start=True, stop=False)
            nc.tensor.matmul(SPm[:, 0:bc], cst["W1pT"][:, 128 * hf:128 * hf + 128],
                             portT[:], start=False, stop=True)
            nc.scalar.activation(ft[:, hf, :], SPm[:, 0:bc],
                                 AF.Relu, bias=cst["b1c"][:, hf:hf + 1])
        ht = constp.tile([128, 2, bc], bf16, tag="ht")
        for hi, w1t, bvec in ((0, "v1T", "bv1"), (1, "a1T", "ba1")):
            SPm = psq.tile([128, 512], fp32, tag="psq", name="SPm2")
            for hf in range(2):
                nc.tensor.matmul(SPm[:, 0:bc], cst[w1t][:, hf, :], ft[:, hf, :],
                                 start=(hf == 0), stop=(hf == 1))
            nc.scalar.activation(ht[:, hi, :], SPm[:, 0:bc],
                                 AF.Relu, bias=cst[bvec][:])
        SPo = psq.tile([128, 512], fp32, tag="psq", name="SPo")
        nc.tensor.matmul(SPo[0:1, 0:bc], cst["v2T"][:], ht[:, 0, :],
                         start=True, stop=True)
        nc.tensor.matmul(SPo[32:32 + NACT, 0:bc], cst["a2T"][:], ht[:, 1, :],
                         start=True, stop=True, tile_position=(0, 32))
        advS = constp.tile([NACT, bc], bf16, tag="advS")
        nc.vector.tensor_copy(advS[:], SPo[32:32 + NACT, 0:bc])
        nc.tensor.matmul(SPo[64:65, 0:bc], cst["ones3"][:], advS[:],
                         start=True, stop=True, tile_position=(0, 64))
        vmS = constp.tile([1, 2, bc], fp32, tag="vmS")
        nc.vector.tensor_copy(vmS[:, 0, :], SPo[0:1, 0:bc])
        nc.vector.tensor_copy(vmS[:, 1, :], SPo[64:65, 0:bc])
        wS = constp.tile([1, bc], fp32, tag="wS")
        nc.vector.tensor_tensor(out=wS[:], in0=vmS[:, 0, :],
                                in1=vmS[:, 1, :], op=ALU.subtract)
        w3 = constp.tile([NACT, bc], fp32, tag="w3")
        nc.gpsimd.partition_broadcast(w3[:], wS[:], channels=NACT)
        advF = constp.tile([NACT, bc], fp32, tag="advF")
        nc.vector.tensor_copy(advF[:], SPo[32:32 + NACT, 0:bc])
        nc.vector.tensor_tensor(out=outT[:], in0=advF[:], in1=w3[:], op=ALU.add)
        nc.vector.tensor_scalar(out=outT[:], in0=outT[:],
                                scalar1=cst["ba2c"][:], scalar2=None, op0=ALU.add)

        # ---------------- store output ------------------------------------
        for half in range((bc + 127) // 128):
            wbc = min(128, bc - 128 * half)
            op = psp.tile([128, 512], fp32, tag="ps", name="op")
            nc.tensor.transpose(op[0:wbc, 0:NACT],
                                outT[:, 128 * half:128 * half + wbc],
                                cst["ident"][0:NACT, 0:NACT])
            os_ = smallp.tile([128, NACT], fp32, tag="os")
            nc.vector.tensor_copy(os_[0:wbc, :], op[0:wbc, 0:NACT])
            nc.sync.dma_start(out=out_d[128 * half:128 * half + wbc, :],
                              in_=os_[0:wbc, :])

    nc.compile()
    return nc


def run_raw(inputs, **spmd_kwargs):
    if "nc" not in _CACHE:
        consts, v2b = _host_prep(inputs)
        _CACHE["consts"] = consts
        _CACHE["nc"] = build_nc(BC, 32, v2b)
    consts, nc = _CACHE["consts"], _CACHE["nc"]

    from concourse.bass_utils import run_bass_kernel_spmd
    state = np.ascontiguousarray(inputs["state"], np.float32)
    in_maps = [dict(consts, state_c=np.ascontiguousarray(state[c * BC:(c + 1) * BC]))
               for c in range(NCORES)]
    return run_bass_kernel_spmd(nc, in_maps, core_ids=list(range(NCORES)),
                                **spmd_kwargs)


def kernel(**inputs):
    res = run_raw(inputs)
    return np.concatenate(
        [res.results[c]["out_c"] for c in range(NCORES)], axis=0).astype(np.float32)
